# revision 9
# baseline (speedup 1.0000x reference)
"""Multi-head attention (B=4, S=2048, D=1024, H=16, Dh=64) on 8 TRN2 NeuronCores.

Sharding: core = (batch, head_group) with 4 batches x 2 head-groups of 8 heads.
Fully data-parallel SPMD - no collectives. Each core computes
out[b, :, hg*512:(hg+1)*512] (returned transposed; host transposes back).

Per-core kernel (fp16 storage, fp32 PSUM accumulation):
  prefix: project K^T (head-dim on partitions), V (key position on
          partitions, with a 65th ones-column per head so the AV matmul
          accumulates the softmax denominator in PSUM row 64 for free),
          and Q^T for q-blocks 0-1.
  attention: flat software pipeline over (qb, quad, kt) steps: scores
          S^T[k,q] per k-tile via row-paired K=64 matmuls (these pipeline
          concurrently in the PE), exp on ScalarE (scale=1/8, bias -4
          everywhere, -1e6 at the masked key row), then two kt-steps later
          AV via M=65 matmuls accumulating [O^T | denom] in one PSUM bank
          per head. Q^T projection for q-blocks 2-3 is injected at early
          quad-pass boundaries. Quad tail: reciprocal of the denom row,
          gpsimd partition-broadcast, per-element scale on VectorE, DMA of
          O^T to DRAM (no transposes on device).
"""

from contextlib import ExitStack

import numpy as np

import concourse.bass as bass
import concourse.bacc as bacc
import concourse.mybir as mybir
import concourse.tile as tile
from concourse.bass_utils import run_bass_kernel_spmd
from concourse.masks import make_identity

B = 4
SEQ = 2048
DM = 1024
H = 16
DH = 64
NCORES = 8
CPC = 512          # output columns per core (8 heads x 64)
P = 128
NQB = SEQ // 512   # q blocks of 512
NKT = SEQ // P     # k tiles of 128
NDT = DM // P      # d_model tiles of 128
HL = 8             # heads per core
LAG = 4            # kt-steps between scores/exp and AV consumption

F32 = mybir.dt.float32
F16 = mybir.dt.float16
EXP = mybir.ActivationFunctionType.Exp

_compiled = None


def _emit(ctx: ExitStack, tc: tile.TileContext, xq, xk, xv, wq, wk, wv, bmask, out_t):
    nc = tc.nc

    small = ctx.enter_context(tc.tile_pool(name="small", bufs=1))
    stage = ctx.enter_context(tc.tile_pool(name="stage", bufs=3))
    qstage = ctx.enter_context(tc.tile_pool(name="qstage", bufs=1))
    wpool = ctx.enter_context(tc.tile_pool(name="wpool", bufs=1))
    proj = ctx.enter_context(tc.tile_pool(name="proj", bufs=1))
    epool = ctx.enter_context(tc.tile_pool(name="epool", bufs=12))
    opool = ctx.enter_context(tc.tile_pool(name="opool", bufs=3))
    ppool = ctx.enter_context(tc.tile_pool(name="ppool", bufs=2))
    ps_sc = ctx.enter_context(tc.tile_pool(name="ps_sc", bufs=2, space="PSUM"))
    ps_av = ctx.enter_context(tc.tile_pool(name="ps_av", bufs=1, space="PSUM"))

    xq_r = xq.ap().rearrange("(dt p) q -> p dt q", p=P)
    xk_r = xk.ap().rearrange("(dt p) q -> p dt q", p=P)
    xv_r = xv.ap().rearrange("(dt p) q -> p dt q", p=P)
    srcs = {"k": xk_r, "v": xv_r, "q": xq_r}

    # ---- startup DMAs, ordered so the first K matmul can start ASAP ----
    w_sb = {}
    w_sb["wk"] = wpool.tile([P, NDT, CPC], F16, tag="wk", name="wk")
    nc.sync.dma_start(w_sb["wk"][:], wk.ap().rearrange("(dt p) c -> p dt c", p=P))

    staged = {}

    def stage_block(kind, blk):
        st = stage.tile([P, NDT, 512], F16, tag="stage", name=f"st_{kind}{blk}")
        nc.sync.dma_start(st[:], srcs[kind][:, :, blk * 512:(blk + 1) * 512])
        staged[(kind, blk)] = st

    order = [("k", b) for b in range(4)] + [("v", b) for b in range(4)] + [("q", 0)]
    stage_block(*order[0])
    stage_block(*order[1])

    w_sb["wv"] = wpool.tile([P, NDT, CPC], F16, tag="wv", name="wv")
    nc.sync.dma_start(w_sb["wv"][:], wv.ap().rearrange("(dt p) c -> p dt c", p=P))
    w_sb["wq"] = wpool.tile([P, NDT, CPC], F16, tag="wq", name="wq")
    nc.sync.dma_start(w_sb["wq"][:], wq.ap().rearrange("(dt p) c -> p dt c", p=P))
    bmask_sb = small.tile([P, NKT], F32)
    nc.sync.dma_start(bmask_sb[:], bmask.ap())
    ident = small.tile([P, P], F32)
    make_identity(nc, ident[:])

    kproj = [proj.tile([P, SEQ], F16, tag=f"kproj{p}", name=f"kproj{p}") for p in range(4)]
    qproj = [proj.tile([P, SEQ], F16, tag=f"qproj{p}", name=f"qproj{p}") for p in range(4)]
    # per head: column 0 is ones (denominator row 0 of the AV product),
    # columns 1-64 are V
    v_sb = proj.tile([P, NKT, HL, 65], F16, tag="v_sb")
    nc.vector.memset(v_sb[:, :, :, 0:1], 1.0)

    pg = [0]  # proj psum tag rotation

    def proj_psum():
        t = ps_av.tile([P, 512], F32, tag=f"av{pg[0] % 4}", name="projps")
        pg[0] += 1
        return t

    def qk_group(wname, st, dst, p, blk):
        ps = proj_psum()
        for dt in range(NDT):
            nc.tensor.matmul(
                ps[:],
                w_sb[wname][:, dt, 128 * p:128 * (p + 1)],
                st[:, dt, :],
                start=(dt == 0),
                stop=(dt == NDT - 1),
            )
        nc.vector.tensor_copy(dst[p][:, blk * 512:(blk + 1) * 512], ps[:])

    # ---- prefix: K (all), V (all), Q (blocks 0-1) ----------------------
    for idx, (kind, blk) in enumerate(order):
        if idx + 2 < len(order):
            stage_block(*order[idx + 2])
        elif idx + 2 == len(order):
            # dedicated staging for q blocks 1-3, consumed mid-attention
            xq_st = {b: qstage.tile([P, NDT, 512], F16, tag=f"xq{b}", name=f"xq{b}")
                     for b in (1, 2, 3)}
            for b in (1, 2, 3):
                nc.sync.dma_start(xq_st[b][:], xq_r[:, :, b * 512:(b + 1) * 512])
        st = staged.pop((kind, blk))
        if kind == "k":
            for p in range(4):
                qk_group("wk", st, kproj, p, blk)
        elif kind == "q":
            for p in range(4):
                qk_group("wq", st, qproj, p, blk)
        else:
            for sub in range(4):
                kt_i = blk * 4 + sub
                ps = proj_psum()
                for dt in range(NDT):
                    nc.tensor.matmul(
                        ps[:],
                        st[:, dt, 128 * sub:128 * (sub + 1)],
                        w_sb["wv"][:, dt, :],
                        start=(dt == 0),
                        stop=(dt == NDT - 1),
                    )
                nc.vector.tensor_copy(v_sb[:, kt_i, :, 1:65], ps[:])

    # ---- attention: flat pipeline --------------------------------------
    NSTEP = NQB * 2 * NKT
    av_tiles = {}
    pending = []
    # (pass boundary index) -> list of (q block, group) to inject there
    qinject = {0: [(1, 0), (1, 1)], 1: [(1, 2), (1, 3)],
               2: [(2, 0), (2, 1)], 3: [(2, 2), (2, 3)],
               4: [(3, 0), (3, 1)], 5: [(3, 2), (3, 3)]}

    def emit_tail(qb, quad):
        av = av_tiles.pop((qb, quad))
        rcp = ppool.tile([P, 16], F32, tag="rcp")
        o_part = ppool.tile([P, 4, 256], F32, tag="opart")
        osb = []
        for i in range(4):
            o = opool.tile([P, 512], F32, tag="osb", name=f"osb{i}")
            nc.vector.tensor_copy(o[0:65, :], av[i][0:65, :])
            osb.append(o)
        for c in range(4):
            for i in range(4):
                nc.tensor.transpose(
                    av[i][:, 65 * c:65 * (c + 1)],
                    osb[i][0:65, 128 * c:128 * (c + 1)],
                    ident[0:65, 0:65],
                )
                j = 4 * i + c
                nc.vector.reciprocal(
                    rcp[:, j:j + 1], av[i][:, 65 * c:65 * c + 1]
                )
                nc.vector.tensor_scalar(
                    o_part[:, c, 64 * i:64 * (i + 1)],
                    av[i][:, 65 * c + 1:65 * c + 65],
                    rcp[:, j:j + 1],
                    None,
                    mybir.AluOpType.mult,
                )
            nc.sync.dma_start(
                out_t.ap()[
                    qb * 512 + c * P:qb * 512 + (c + 1) * P,
                    quad * 256:(quad + 1) * 256,
                ],
                o_part[:, c, :],
            )

    for step in range(NSTEP + LAG):
        if step < NSTEP:
            g = step // NKT
            qb, quad, kt = g // 2, g % 2, step % NKT
            qs = slice(qb * 512, (qb + 1) * 512)
            es = []
            for pi in range(2):
                pr = 2 * quad + pi
                st_ps = ps_sc.tile([P, 1024], F32, tag="scores")
                for hh in range(2):
                    rows = slice(64 * hh, 64 * (hh + 1))
                    nc.tensor.matmul(
                        st_ps[:, 512 * hh:512 * (hh + 1)],
                        kproj[pr][rows, kt * P:(kt + 1) * P],
                        qproj[pr][rows, qs],
                        start=True,
                        stop=True,
                        tile_position=(64 * hh, 0),
                    )
                e = epool.tile([P, 1024], F16, tag="e")
                nc.scalar.activation(
                    e[:], st_ps[:], EXP,
                    bias=bmask_sb[:, kt:kt + 1], scale=0.125,
                )
                es.append(e)
            pending.append((qb, quad, kt, es))
        if step >= LAG:
            qb_b, quad_b, kt_b, es_b = pending.pop(0)
            if kt_b == 0:
                av_tiles[(qb_b, quad_b)] = [
                    ps_av.tile([P, 512], F32, tag=f"av{i}", name=f"av{i}")
                    for i in range(4)
                ]
            av = av_tiles[(qb_b, quad_b)]
            for pi in range(2):
                for hh in range(2):
                    i = 2 * pi + hh
                    hl = 4 * quad_b + i
                    nc.tensor.matmul(
                        av[i][0:65, :],
                        v_sb[:, kt_b, hl, :],
                        es_b[pi][:, 512 * hh:512 * (hh + 1)],
                        start=(kt_b == 0),
                        stop=(kt_b == NKT - 1),
                    )
            if kt_b == NKT - 1:
                emit_tail(qb_b, quad_b)
                pidx = 2 * qb_b + quad_b
                for qb_i, g_i in qinject.get(pidx, ()):
                    qk_group("wq", xq_st[qb_i], qproj, g_i, qb_i)


def build():
    global _compiled
    if _compiled is not None:
        return _compiled
    nc = bacc.Bacc("TRN2", target_bir_lowering=False, debug=False)
    xq = nc.dram_tensor("xq", [DM, SEQ], F16, kind="ExternalInput")
    xk = nc.dram_tensor("xk", [DM, SEQ], F16, kind="ExternalInput")
    xv = nc.dram_tensor("xv", [DM, SEQ], F16, kind="ExternalInput")
    wq = nc.dram_tensor("wq", [DM, CPC], F16, kind="ExternalInput")
    wk = nc.dram_tensor("wk", [DM, CPC], F16, kind="ExternalInput")
    wv = nc.dram_tensor("wv", [DM, CPC], F16, kind="ExternalInput")
    bmask = nc.dram_tensor("bmask", [P, NKT], F32, kind="ExternalInput")
    out_t = nc.dram_tensor("out_t", [SEQ, CPC], F32, kind="ExternalOutput")
    with tile.TileContext(nc) as tc:
        with ExitStack() as ctx:
            _emit(ctx, tc, xq, xk, xv, wq, wk, wv, bmask, out_t)
    nc.compile()
    _compiled = nc
    return nc


def make_in_maps(Q_seq, K_seq, V_seq, V_len, WQ, WK, WV):
    in_maps = []
    for core in range(NCORES):
        b, hg = divmod(core, 2)
        cols = slice(hg * CPC, (hg + 1) * CPC)
        bm = np.full((P, NKT), -4.0, np.float32)
        vl = int(V_len[b, 0])
        bm[vl % P, vl // P] = -1e6
        in_maps.append(
            {
                "xq": np.ascontiguousarray(Q_seq[b].T).astype(np.float16),
                "xk": np.ascontiguousarray(K_seq[b].T).astype(np.float16),
                "xv": np.ascontiguousarray(V_seq[b].T).astype(np.float16),
                "wq": np.ascontiguousarray(WQ[:, cols]).astype(np.float16),
                "wk": np.ascontiguousarray(WK[:, cols]).astype(np.float16),
                "wv": np.ascontiguousarray(WV[:, cols]).astype(np.float16),
                "bmask": bm,
            }
        )
    return in_maps


def kernel(Q_seq, K_seq, V_seq, Q_len, V_len, WQ, WK, WV, _trace=False):
    nc = build()
    in_maps = make_in_maps(Q_seq, K_seq, V_seq, V_len, WQ, WK, WV)
    res = run_bass_kernel_spmd(
        nc, in_maps, core_ids=list(range(NCORES)), trace=_trace
    )
    out = np.empty((B, SEQ, H * DH), np.float32)
    for core in range(NCORES):
        b, hg = divmod(core, 2)
        out[b, :, hg * CPC:(hg + 1) * CPC] = res.results[core]["out_t"]
    for b in range(B):
        out[b, int(Q_len[b, 0]), :] = 0.0
    if _trace:
        kernel._last_results = res
    return out


# revision 10
# speedup vs baseline: 1.1504x; 1.1504x over previous
"""Multi-head attention (B=4, S=2048, D=1024, H=16, Dh=64) on 8 TRN2 NeuronCores.

Sharding: core = (batch, head_group) with 4 batches x 2 head-groups of 8 heads.
Fully data-parallel SPMD - no collectives. Each core computes
out[b, :, hg*512:(hg+1)*512] (returned transposed; host transposes back).

Per-core kernel (fp16 storage, fp32 PSUM accumulation):
  prefix: project K^T (head-dim on partitions), V (key position on
          partitions, with a 65th ones-column per head so the AV matmul
          accumulates the softmax denominator in PSUM row 64 for free),
          and Q^T for q-blocks 0-1.
  attention: flat software pipeline over (qb, quad, kt) steps: scores
          S^T[k,q] per k-tile via row-paired K=64 matmuls (these pipeline
          concurrently in the PE), exp on ScalarE (scale=1/8, bias -4
          everywhere, -1e6 at the masked key row), then two kt-steps later
          AV via M=65 matmuls accumulating [O^T | denom] in one PSUM bank
          per head. Q^T projection for q-blocks 2-3 is injected at early
          quad-pass boundaries. Quad tail: reciprocal of the denom row,
          gpsimd partition-broadcast, per-element scale on VectorE, DMA of
          O^T to DRAM (no transposes on device).
"""

from contextlib import ExitStack

import numpy as np

import concourse.bass as bass
import concourse.bacc as bacc
import concourse.mybir as mybir
import concourse.tile as tile
from concourse.bass_utils import run_bass_kernel_spmd
from concourse.masks import make_identity

B = 4
SEQ = 2048
DM = 1024
H = 16
DH = 64
NCORES = 8
CPC = 512          # output columns per core (8 heads x 64)
P = 128
NQB = SEQ // 512   # q blocks of 512
NKT = SEQ // P     # k tiles of 128
NDT = DM // P      # d_model tiles of 128
HL = 8             # heads per core
LAG = 2            # kt-steps between scores/exp and AV consumption

F32 = mybir.dt.float32
F16 = mybir.dt.float16
EXP = mybir.ActivationFunctionType.Exp

_compiled = None


def _emit(ctx: ExitStack, tc: tile.TileContext, xq, xk, xv, wq, wk, wv, bmask, out_t):
    nc = tc.nc

    small = ctx.enter_context(tc.tile_pool(name="small", bufs=1))
    stage = ctx.enter_context(tc.tile_pool(name="stage", bufs=3))
    qstage = ctx.enter_context(tc.tile_pool(name="qstage", bufs=1))
    wpool = ctx.enter_context(tc.tile_pool(name="wpool", bufs=1))
    proj = ctx.enter_context(tc.tile_pool(name="proj", bufs=1))
    epool = ctx.enter_context(tc.tile_pool(name="epool", bufs=12))
    opool = ctx.enter_context(tc.tile_pool(name="opool", bufs=3))
    ppool = ctx.enter_context(tc.tile_pool(name="ppool", bufs=2))
    ps_sc = ctx.enter_context(tc.tile_pool(name="ps_sc", bufs=2, space="PSUM"))
    ps_av = ctx.enter_context(tc.tile_pool(name="ps_av", bufs=1, space="PSUM"))

    xq_r = xq.ap().rearrange("(dt p) q -> p dt q", p=P)
    xk_r = xk.ap().rearrange("(dt p) q -> p dt q", p=P)
    xv_r = xv.ap().rearrange("(dt p) q -> p dt q", p=P)
    srcs = {"k": xk_r, "v": xv_r, "q": xq_r}

    # ---- startup DMAs, ordered so the first K matmul can start ASAP ----
    w_sb = {}
    w_sb["wk"] = wpool.tile([P, NDT, CPC], F16, tag="wk", name="wk")
    nc.sync.dma_start(w_sb["wk"][:], wk.ap().rearrange("(dt p) c -> p dt c", p=P))

    staged = {}

    def stage_block(kind, blk):
        st = stage.tile([P, NDT, 512], F16, tag="stage", name=f"st_{kind}{blk}")
        nc.sync.dma_start(st[:], srcs[kind][:, :, blk * 512:(blk + 1) * 512])
        staged[(kind, blk)] = st

    order = [("k", b) for b in range(4)] + [("v", b) for b in range(4)] + [("q", 0)]
    stage_block(*order[0])
    stage_block(*order[1])

    w_sb["wv"] = wpool.tile([P, NDT, CPC], F16, tag="wv", name="wv")
    nc.sync.dma_start(w_sb["wv"][:], wv.ap().rearrange("(dt p) c -> p dt c", p=P))
    w_sb["wq"] = wpool.tile([P, NDT, CPC], F16, tag="wq", name="wq")
    nc.sync.dma_start(w_sb["wq"][:], wq.ap().rearrange("(dt p) c -> p dt c", p=P))
    bmask_sb = small.tile([P, NKT], F32)
    nc.sync.dma_start(bmask_sb[:], bmask.ap())
    ident = small.tile([P, P], F32)
    make_identity(nc, ident[:])

    kproj = [proj.tile([P, SEQ], F16, tag=f"kproj{p}", name=f"kproj{p}") for p in range(4)]
    qproj = [proj.tile([P, SEQ], F16, tag=f"qproj{p}", name=f"qproj{p}") for p in range(4)]
    # per head: column 0 is ones (denominator row 0 of the AV product),
    # columns 1-64 are V
    v_sb = proj.tile([P, NKT, HL, 65], F16, tag="v_sb")
    nc.vector.memset(v_sb[:, :, :, 0:1], 1.0)

    pg = [0]  # proj psum tag rotation

    def proj_psum():
        t = ps_av.tile([P, 512], F32, tag=f"av{pg[0] % 4}", name="projps")
        pg[0] += 1
        return t

    def qk_group(wname, st, dst, p, blk):
        ps = proj_psum()
        for dt in range(NDT):
            nc.tensor.matmul(
                ps[:],
                w_sb[wname][:, dt, 128 * p:128 * (p + 1)],
                st[:, dt, :],
                start=(dt == 0),
                stop=(dt == NDT - 1),
            )
        nc.vector.tensor_copy(dst[p][:, blk * 512:(blk + 1) * 512], ps[:])

    # ---- prefix: K (all), V (all), Q (blocks 0-1) ----------------------
    for idx, (kind, blk) in enumerate(order):
        if idx + 2 < len(order):
            stage_block(*order[idx + 2])
        elif idx + 2 == len(order):
            # dedicated staging for q blocks 1-3, consumed mid-attention
            xq_st = {b: qstage.tile([P, NDT, 512], F16, tag=f"xq{b}", name=f"xq{b}")
                     for b in (1, 2, 3)}
            for b in (1, 2, 3):
                nc.sync.dma_start(xq_st[b][:], xq_r[:, :, b * 512:(b + 1) * 512])
        st = staged.pop((kind, blk))
        if kind == "k":
            for p in range(4):
                qk_group("wk", st, kproj, p, blk)
        elif kind == "q":
            for p in range(4):
                qk_group("wq", st, qproj, p, blk)
        else:
            for sub in range(4):
                kt_i = blk * 4 + sub
                ps = proj_psum()
                for dt in range(NDT):
                    nc.tensor.matmul(
                        ps[:],
                        st[:, dt, 128 * sub:128 * (sub + 1)],
                        w_sb["wv"][:, dt, :],
                        start=(dt == 0),
                        stop=(dt == NDT - 1),
                    )
                nc.vector.tensor_copy(v_sb[:, kt_i, :, 1:65], ps[:])

    # ---- attention: flat pipeline --------------------------------------
    NSTEP = NQB * 2 * NKT
    av_tiles = {}
    pending = []
    # (pass boundary index) -> list of (q block, group) to inject there
    qinject = {0: [(1, 0), (1, 1)], 1: [(1, 2), (1, 3)],
               2: [(2, 0), (2, 1)], 3: [(2, 2), (2, 3)],
               4: [(3, 0), (3, 1)], 5: [(3, 2), (3, 3)]}

    def emit_tail(qb, quad):
        av = av_tiles.pop((qb, quad))
        rcp = ppool.tile([P, 16], F32, tag="rcp")
        o_part = ppool.tile([P, 4, 256], F32, tag="opart")
        osb = []
        for i in range(4):
            o = opool.tile([P, 512], F32, tag="osb", name=f"osb{i}")
            nc.vector.tensor_copy(o[0:65, :], av[i][0:65, :])
            osb.append(o)
        for c in range(4):
            for i in range(4):
                nc.tensor.transpose(
                    av[i][:, 65 * c:65 * (c + 1)],
                    osb[i][0:65, 128 * c:128 * (c + 1)],
                    ident[0:65, 0:65],
                )
                j = 4 * i + c
                nc.vector.reciprocal(
                    rcp[:, j:j + 1], av[i][:, 65 * c:65 * c + 1]
                )
                nc.vector.tensor_scalar(
                    o_part[:, c, 64 * i:64 * (i + 1)],
                    av[i][:, 65 * c + 1:65 * c + 65],
                    rcp[:, j:j + 1],
                    None,
                    mybir.AluOpType.mult,
                )
            nc.sync.dma_start(
                out_t.ap()[
                    qb * 512 + c * P:qb * 512 + (c + 1) * P,
                    quad * 256:(quad + 1) * 256,
                ],
                o_part[:, c, :],
            )

    for step in range(NSTEP + LAG):
        if step < NSTEP:
            g = step // NKT
            qb, quad, kt = g // 2, g % 2, step % NKT
            qs = slice(qb * 512, (qb + 1) * 512)
            es = []
            for pi in range(2):
                pr = 2 * quad + pi
                st_ps = ps_sc.tile([P, 1024], F32, tag="scores")
                for hh in range(2):
                    rows = slice(64 * hh, 64 * (hh + 1))
                    nc.tensor.matmul(
                        st_ps[:, 512 * hh:512 * (hh + 1)],
                        kproj[pr][rows, kt * P:(kt + 1) * P],
                        qproj[pr][rows, qs],
                        start=True,
                        stop=True,
                        tile_position=(64 * hh, 0),
                    )
                e = epool.tile([P, 1024], F16, tag="e")
                nc.scalar.activation(
                    e[:], st_ps[:], EXP,
                    bias=bmask_sb[:, kt:kt + 1], scale=0.125,
                )
                es.append(e)
            pending.append((qb, quad, kt, es))
        if step >= LAG:
            qb_b, quad_b, kt_b, es_b = pending.pop(0)
            if kt_b == 0:
                av_tiles[(qb_b, quad_b)] = [
                    ps_av.tile([P, 512], F32, tag=f"av{i}", name=f"av{i}")
                    for i in range(4)
                ]
            av = av_tiles[(qb_b, quad_b)]
            for pi in range(2):
                for hh in range(2):
                    i = 2 * pi + hh
                    hl = 4 * quad_b + i
                    nc.tensor.matmul(
                        av[i][0:65, :],
                        v_sb[:, kt_b, hl, :],
                        es_b[pi][:, 512 * hh:512 * (hh + 1)],
                        start=(kt_b == 0),
                        stop=(kt_b == NKT - 1),
                    )
            if kt_b == NKT - 1:
                emit_tail(qb_b, quad_b)
                pidx = 2 * qb_b + quad_b
                for qb_i, g_i in qinject.get(pidx, ()):
                    qk_group("wq", xq_st[qb_i], qproj, g_i, qb_i)


def build():
    global _compiled
    if _compiled is not None:
        return _compiled
    nc = bacc.Bacc("TRN2", target_bir_lowering=False, debug=False)
    xq = nc.dram_tensor("xq", [DM, SEQ], F16, kind="ExternalInput")
    xk = nc.dram_tensor("xk", [DM, SEQ], F16, kind="ExternalInput")
    xv = nc.dram_tensor("xv", [DM, SEQ], F16, kind="ExternalInput")
    wq = nc.dram_tensor("wq", [DM, CPC], F16, kind="ExternalInput")
    wk = nc.dram_tensor("wk", [DM, CPC], F16, kind="ExternalInput")
    wv = nc.dram_tensor("wv", [DM, CPC], F16, kind="ExternalInput")
    bmask = nc.dram_tensor("bmask", [P, NKT], F32, kind="ExternalInput")
    out_t = nc.dram_tensor("out_t", [SEQ, CPC], F32, kind="ExternalOutput")
    with tile.TileContext(nc) as tc:
        with ExitStack() as ctx:
            _emit(ctx, tc, xq, xk, xv, wq, wk, wv, bmask, out_t)
    nc.compile()
    _compiled = nc
    return nc


def make_in_maps(Q_seq, K_seq, V_seq, V_len, WQ, WK, WV):
    in_maps = []
    for core in range(NCORES):
        b, hg = divmod(core, 2)
        cols = slice(hg * CPC, (hg + 1) * CPC)
        bm = np.full((P, NKT), -4.0, np.float32)
        vl = int(V_len[b, 0])
        bm[vl % P, vl // P] = -1e6
        in_maps.append(
            {
                "xq": np.ascontiguousarray(Q_seq[b].T).astype(np.float16),
                "xk": np.ascontiguousarray(K_seq[b].T).astype(np.float16),
                "xv": np.ascontiguousarray(V_seq[b].T).astype(np.float16),
                "wq": np.ascontiguousarray(WQ[:, cols]).astype(np.float16),
                "wk": np.ascontiguousarray(WK[:, cols]).astype(np.float16),
                "wv": np.ascontiguousarray(WV[:, cols]).astype(np.float16),
                "bmask": bm,
            }
        )
    return in_maps


def kernel(Q_seq, K_seq, V_seq, Q_len, V_len, WQ, WK, WV, _trace=False):
    nc = build()
    in_maps = make_in_maps(Q_seq, K_seq, V_seq, V_len, WQ, WK, WV)
    res = run_bass_kernel_spmd(
        nc, in_maps, core_ids=list(range(NCORES)), trace=_trace
    )
    out = np.empty((B, SEQ, H * DH), np.float32)
    for core in range(NCORES):
        b, hg = divmod(core, 2)
        out[b, :, hg * CPC:(hg + 1) * CPC] = res.results[core]["out_t"]
    for b in range(B):
        out[b, int(Q_len[b, 0]), :] = 0.0
    if _trace:
        kernel._last_results = res
    return out
